# revision 46
# baseline (speedup 1.0000x reference)
"""Trainium2 Bass kernel for nn_AttentionSublayer (B=4, T=1024, D=1024, H=16, DH=64, L=128).

End-to-end (wall-clock) optimized: the axon tunnel moves ~60 MB/s, so the
dominant costs are host<->device bytes and per-call dispatch, not FLOPs.

Sharding: 8 cores = 4 batches x 2 query-parity groups. Core c handles batch
b=c//2 and the interleaved query subset q_global = 2*q_local + th (th=c%2),
ALL 16 heads. Each core therefore:
  - projects Q for its 512 queries, K/V for all 1024 keys (K/V projection is
    computed redundantly by the pair of cores sharing a batch),
  - computes scoresT[k, q_local] = K.Q + pos, exp, attn@V, normalize,
  - emits a disjoint (512, 1024) bf16 slice of y (rows q_global).

Relative-position logits use the skewed-gather trick: P[q, r] = Q[q].T_core[r]
is written to DRAM scratch, and score tiles gather P along the diagonal with
partition stride EW-2 (slope 2 because queries are interleaved). The per-core
parity shift th is absorbed into the host-built table T_core[r] =
pos_emb[clip(r - C - th, -L, L) + L], so the instruction stream is identical
on every core (true SPMD). Tiles fully outside the gather window are filled
with rank-1 (ones x sat_row) matmuls using the clamped table edges.

Transfers are bf16 and content-hash cached: weight/x/pos uploads are kept
device-resident across calls and re-verified by sha256, so repeat calls with
unchanged inputs only pay dispatch + the 8 MB y readback.
"""

import hashlib

import numpy as np
import ml_dtypes

import jax
from jax.sharding import Mesh, PartitionSpec, NamedSharding

# Strip absolute source paths from lowered HLO so the persistent NEFF
# compile cache (keyed on HLO bytes) hits regardless of the directory this
# file runs from.
jax.config.update("jax_hlo_source_file_canonicalization_regex", ".*")

import concourse.bass as bass
import concourse.bacc as bacc
import concourse.mybir as mybir
import concourse.tile as tile
from concourse import bass2jax

BF16 = ml_dtypes.bfloat16
B, T, D, H, DH, L = 4, 1024, 1024, 16, 64, 128
SCALE = 8.0
NCORES = 8
TQ = T // 2          # queries per core (interleaved)
NEG = -30000.0
FP = mybir.dt.float32
FR = mybir.dt.float32r
BF = mybir.dt.bfloat16

EW = 768             # expanded pos-table width
C0 = 382             # table center offset: r = v + C0 + th
QT_TILES = TQ // 128  # 4
KT_TILES = T // 128   # 8
DT_TILES = D // 128   # 8

# flat element offsets inside the per-core bf16 x blob
XQ_OFF = 0
XK_OFF = TQ * D                 # 524288
XV_OFF = XK_OFF + T * D         # 1572864
XBN = XV_OFF + T * D            # 2621440

WBN = 4 * D * D                 # wqT | wkT | wvT | woT

# fp32 table blob: TBL (128, EW) | SAT (128, 2) | identf (128, 128)
TBL_OFF = 0
SAT_OFF = 128 * EW              # 98304
IDF_OFF = SAT_OFF + 256         # 98560
TBN = IDF_OFF + 128 * 128       # 114944

# bf16 const blob: identb (128, 128) | ones (128, 16)
CB_ONES_OFF = 128 * 128
CBN = CB_ONES_OFF + 128 * 16


def fr(ap):
    return ap.bitcast(FR)


def _canonicalize_bir(raw):
    """Strip run-directory paths and tracebacks from the serialized BIR so the
    HLO built from it (which keys the persistent NEFF compile cache) is
    byte-stable regardless of the directory this file runs from."""
    import json
    import os.path
    m = json.loads(raw)

    def walk(o):
        if isinstance(o, dict):
            if isinstance(o.get("filename"), str):
                o["filename"] = os.path.basename(o["filename"])
            if "ant_traceback" in o:
                o["ant_traceback"] = None
            for v in o.values():
                walk(v)
        elif isinstance(o, list):
            for v in o:
                walk(v)

    walk(m)
    return json.dumps(m, separators=(",", ":")).encode()


def build_nc():
    nc = bacc.Bacc("TRN2", target_bir_lowering=False, debug=False,
                   num_devices=NCORES)

    xbd = nc.dram_tensor("xb", (XBN,), BF, kind="ExternalInput").ap()
    wbd = nc.dram_tensor("wb", (WBN,), BF, kind="ExternalInput").ap()
    tbd = nc.dram_tensor("tb", (TBN,), FR, kind="ExternalInput").ap()
    mbd = nc.dram_tensor("mbk", (T,), FP, kind="ExternalInput").ap()
    cbd = nc.dram_tensor("cb", (CBN,), BF, kind="ExternalInput").ap()
    yd = nc.dram_tensor("y", (TQ, D), BF, kind="ExternalOutput").ap()

    def dap(base, off, dims):
        return bass.AP(base.tensor, off, dims)

    with tile.TileContext(nc) as tc:
        with (
            tc.tile_pool(name="pers", bufs=1) as pers,
            tc.tile_pool(name="dram", bufs=1, space="DRAM") as dpool,
        ):
            # ---- persistent SBUF ----
            QT = [pers.tile([128, TQ], FR, tag=f"qt{i}", name=f"qt{i}") for i in range(8)]
            KT = [pers.tile([128, T], FR, tag=f"kt{i}", name=f"kt{i}") for i in range(8)]
            VA = [pers.tile([128, H * 65], BF, tag=f"va{i}", name=f"va{i}") for i in range(KT_TILES)]
            HT = [pers.tile([128, TQ], BF, tag=f"ht{i}", name=f"ht{i}") for i in range(8)]
            WO = [pers.tile([128, D], BF, tag=f"wo{i}", name=f"wo{i}") for i in range(8)]
            TBL = pers.tile([128, EW], FR, tag="tbl", name="tbl")
            SAT = pers.tile([128, 2], FR, tag="sat", name="sat")
            MB = pers.tile([128, KT_TILES], FP, tag="mb", name="mb")
            IDF = pers.tile([128, 128], FP, tag="idf", name="idf")
            IDB = pers.tile([128, 128], BF, tag="idb", name="idb")
            ONESB = pers.tile([1, 128], BF, tag="onesb", name="onesb")
            ONES65 = pers.tile([65, 64], FP, tag="ones65", name="ones65")

            nc.sync.dma_start(out=TBL[:, :], in_=dap(tbd, TBL_OFF, [[EW, 128], [1, EW]]))
            nc.sync.dma_start(out=SAT[:, :], in_=dap(tbd, SAT_OFF, [[2, 128], [1, 2]]))
            nc.sync.dma_start(out=IDF[:, :], in_=dap(tbd, IDF_OFF, [[128, 128], [1, 128]]).bitcast(FP))
            nc.sync.dma_start(out=IDB[:, :], in_=dap(cbd, 0, [[128, 128], [1, 128]]))
            # mask bias: flat (T,) -> (128 part, kt free)
            nc.sync.dma_start(out=MB[:, :], in_=dap(mbd, 0, [[1, 128], [128, KT_TILES]]))
            nc.sync.dma_start(
                out=ONESB[:, :],
                in_=bass.AP(cbd.tensor, CB_ONES_OFF, [[128, 1], [1, 128]]))
            nc.vector.memset(ONES65[64:65, :], 1.0)
            for kt in range(KT_TILES):
                nc.sync.dma_start(
                    out=VA[kt][:, :].rearrange("p (h c) -> p h c", h=H)[:, :, 64:65],
                    in_=bass.AP(cbd.tensor, CB_ONES_OFF, [[16, 128], [1, 16], [1, 1]]))

            dh = [dpool.tile([TQ, EW], FP, tag=f"dh{h}", name=f"dh{h}") for h in range(H)]

            # ================= Phase A: projections =================
            with (
                tc.tile_pool(name="wsb", bufs=1) as wsb,
                tc.tile_pool(name="xin", bufs=1) as xin,
                tc.tile_pool(name="xtp", bufs=1) as xtp,
                tc.tile_pool(name="ps_a", bufs=2, space="PSUM") as ps_a,
                tc.tile_pool(name="ps_t", bufs=4, space="PSUM") as ps_t,
            ):
                WQ = [wsb.tile([128, D], BF, tag=f"wq{d}", name=f"wq{d}") for d in range(8)]
                WK = [wsb.tile([128, D], BF, tag=f"wk{d}", name=f"wk{d}") for d in range(8)]
                WV = [wsb.tile([128, D], BF, tag=f"wv{d}", name=f"wv{d}") for d in range(8)]
                for d in range(8):
                    nc.sync.dma_start(out=WQ[d][:, :], in_=dap(wbd, 0 * D * D + d * 128 * D, [[D, 128], [1, D]]))
                    nc.sync.dma_start(out=WK[d][:, :], in_=dap(wbd, 1 * D * D + d * 128 * D, [[D, 128], [1, D]]))
                    nc.sync.dma_start(out=WV[d][:, :], in_=dap(wbd, 2 * D * D + d * 128 * D, [[D, 128], [1, D]]))
                    nc.sync.dma_start(out=WO[d][:, :], in_=dap(wbd, 3 * D * D + d * 128 * D, [[D, 128], [1, D]]))

                xT = [xtp.tile([128, T], BF, tag=f"xt{d}", name=f"xt{d}") for d in range(8)]

                def load_and_transpose(xoff, ntt):
                    """Load natural (tok, d) tiles and PE-transpose into xT[(d, tok)]."""
                    xn = [xin.tile([128, D], BF, tag=f"xn{t}", name=f"xn{t}") for t in range(ntt)]
                    for t in range(ntt):
                        nc.sync.dma_start(
                            out=xn[t][:, :],
                            in_=dap(xbd, xoff + t * 128 * D, [[D, 128], [1, D]]))
                    for t in range(ntt):
                        for d in range(8):
                            pt = ps_t.tile([128, 128], BF, tag="pt", name="pt")
                            nc.tensor.transpose(pt[:, :], xn[t][:, d * 128:(d + 1) * 128], IDB[:, :])
                            if (t + d) % 2 == 0:
                                nc.vector.tensor_copy(xT[d][:, t * 128:(t + 1) * 128], pt[:, :])
                            else:
                                nc.scalar.copy(xT[d][:, t * 128:(t + 1) * 128], pt[:, :])

                # --- Q (512 queries) ---
                load_and_transpose(XQ_OFF, QT_TILES)
                for ct in range(8):
                    ps = ps_a.tile([128, T], FP, tag="pj", name="pj")
                    for d in range(8):
                        nc.tensor.matmul(
                            ps[:, 0:TQ], WQ[d][:, ct * 128:(ct + 1) * 128], xT[d][:, 0:TQ],
                            start=(d == 0), stop=(d == 7))
                    nc.vector.tensor_copy(QT[ct][:, :], ps[:, 0:TQ])

                # --- K (all 1024 keys) ---
                load_and_transpose(XK_OFF, KT_TILES)
                for ct in range(8):
                    ps = ps_a.tile([128, T], FP, tag="pj", name="pj")
                    for c in range(2):
                        sl = slice(c * 512, (c + 1) * 512)
                        for d in range(8):
                            nc.tensor.matmul(
                                ps[:, sl], WK[d][:, ct * 128:(ct + 1) * 128], xT[d][:, sl],
                                start=(d == 0), stop=(d == 7))
                    nc.vector.tensor_copy(KT[ct][:, :], ps[:, :])

                # --- V natural (tok-part), packed per-head into VA with ones col ---
                load_and_transpose(XV_OFF, KT_TILES)
                for kt in range(KT_TILES):
                    ps = ps_a.tile([128, T], FP, tag="pj", name="pj")
                    for c in range(2):
                        sl = slice(c * 512, (c + 1) * 512)
                        for d in range(8):
                            nc.tensor.matmul(
                                ps[:, sl], xT[d][:, kt * 128:(kt + 1) * 128], WV[d][:, sl],
                                start=(d == 0), stop=(d == 7))
                    src = ps[:, :].rearrange("p (h c) -> p h c", h=H)
                    dst = VA[kt][:, :].rearrange("p (h c) -> p h c", h=H)[:, :, 0:64]
                    nc.vector.tensor_copy(dst, src)

            # ================= Phase B: attention per head =================
            # score-tile classification (s = 128*(kt - 2*qt), th-independent):
            #   gather  : kt - 2*qt in {-1, 0, 1, 2}
            #   sat-lo  : kt - 2*qt <= -2   (v <= -129 for both th)
            #   sat-hi  : kt - 2*qt >= 3    (v >= +129 for both th)
            with (
                tc.tile_pool(name="pqs", bufs=2) as pqs_pool,
                tc.tile_pool(name="gt", bufs=4) as gpool,
                tc.tile_pool(name="satp", bufs=2) as satp,
                tc.tile_pool(name="expp", bufs=2) as expp,
                tc.tile_pool(name="oaux", bufs=2) as oaux,
                tc.tile_pool(name="ps_sc", bufs=2, space="PSUM") as ps_sc,
                tc.tile_pool(name="ps_pqe", bufs=1, space="PSUM") as ps_pqe,
                tc.tile_pool(name="ps_oa", bufs=2, space="PSUM") as ps_oa,
            ):
                for h in range(H):
                    hi, p0 = h // 2, (h % 2) * 64
                    qsl = QT[hi][p0:p0 + 64, :]      # (64, TQ) fp32
                    ksl = KT[hi][p0:p0 + 64, :]      # (64, T)  fp32

                    # --- saturated rows (bf16): satlo = Q.pos[0], sathi = Q.pos[256]
                    satlo = satp.tile([1, TQ], BF, tag="satlo", name="satlo")
                    sathi = satp.tile([1, TQ], BF, tag="sathi", name="sathi")
                    for j, dstt in ((0, satlo), (1, sathi)):
                        pss = ps_oa.tile([65, TQ], FP, tag="oa", name="pss")
                        nc.tensor.matmul(
                            pss[0:1, :], fr(SAT[p0:p0 + 64, j:j + 1]), fr(qsl),
                            start=True, stop=True)
                        nc.vector.tensor_copy(dstt[:, :], pss[0:1, :])

                    # --- P[q, r] = Q.T_core[r]  -> DRAM dh[h]
                    for qt in range(QT_TILES):
                        pqe = ps_pqe.tile([128, EW], FP, tag="pqe", name="pqe")
                        nc.tensor.matmul(
                            pqe[:, 0:512], fr(qsl[:, qt * 128:(qt + 1) * 128]),
                            fr(TBL[p0:p0 + 64, 0:512]), start=True, stop=True)
                        nc.tensor.matmul(
                            pqe[:, 512:EW], fr(qsl[:, qt * 128:(qt + 1) * 128]),
                            fr(TBL[p0:p0 + 64, 512:EW]), start=True, stop=True)
                        pq = pqs_pool.tile([128, EW], FP, tag="pqs", name="pqs")
                        nc.vector.tensor_copy(pq[:, :], pqe[:, :])
                        nc.sync.dma_start(out=dh[h][qt * 128:(qt + 1) * 128, :], in_=pq[:, :])

                    # --- scores per k-tile + exp
                    ex = [expp.tile([128, TQ], BF, tag=f"ex{kt}", name=f"ex{kt}") for kt in range(KT_TILES)]
                    for kt in range(KT_TILES):
                        sc = ps_sc.tile([128, TQ], FP, tag="sc", name="sc")
                        ops = [("qk",)]
                        lo_qts = [qt for qt in range(QT_TILES) if kt - 2 * qt <= -2]
                        hi_qts = [qt for qt in range(QT_TILES) if kt - 2 * qt >= 3]
                        if lo_qts:
                            ops.append(("r1", satlo, min(lo_qts) * 128, (max(lo_qts) + 1) * 128))
                        if hi_qts:
                            ops.append(("r1", sathi, min(hi_qts) * 128, (max(hi_qts) + 1) * 128))
                        for qt in range(QT_TILES):
                            if kt - 2 * qt in (-1, 0, 1, 2):
                                ops.append(("g", qt))
                        n = len(ops)
                        for i, op in enumerate(ops):
                            st, sp = (i == 0), (i == n - 1)
                            if op[0] == "qk":
                                nc.tensor.matmul(
                                    sc[:, :], fr(ksl[:, kt * 128:(kt + 1) * 128]), fr(qsl),
                                    start=st, stop=sp)
                            elif op[0] == "r1":
                                _, row, s0, s1 = op
                                nc.tensor.matmul(
                                    sc[:, s0:s1], ONESB[0:1, :], row[0:1, s0:s1],
                                    start=st, stop=sp)
                            else:
                                qt = op[1]
                                g = gpool.tile([128, 128], FP, tag="g", name="g")
                                off = qt * 128 * (EW - 2) + C0 + 128 * kt
                                nc.sync.dma_start(
                                    out=g[:, :],
                                    in_=bass.AP(dh[h][:, :].tensor, off, [[EW - 2, 128], [1, 128]]))
                                nc.tensor.matmul(
                                    sc[:, qt * 128:(qt + 1) * 128], g[:, :], IDF[:, :],
                                    is_transpose=True, start=st, stop=sp)
                        nc.scalar.activation(
                            ex[kt][:, :], sc[:, :],
                            mybir.ActivationFunctionType.Exp,
                            bias=MB[:, kt:kt + 1], scale=1.0 / SCALE)

                    # --- attn @ V_aug -> (65, TQ): row 64 = denominator
                    oa = ps_oa.tile([65, TQ], FP, tag="oa", name="oa")
                    for kt in range(KT_TILES):
                        nc.tensor.matmul(
                            oa[:, :], VA[kt][:, h * 65:(h + 1) * 65], ex[kt][:, :],
                            start=(kt == 0), stop=(kt == KT_TILES - 1))
                    os_ = oaux.tile([65, TQ], FP, tag="os", name="os")
                    nc.vector.tensor_copy(os_[:, :], oa[:, :])

                    # --- normalize: PE-replicate denominator, recip, mult -> HT
                    rp = ps_oa.tile([65, TQ], FP, tag="oa", name="rp")
                    nc.tensor.matmul(
                        rp[0:64, :], ONES65[64:65, :], os_[64:65, :],
                        start=True, stop=True)
                    rec = oaux.tile([64, TQ], FP, tag="rec", name="rec")
                    nc.vector.reciprocal(rec[:, :], rp[0:64, :])
                    nc.vector.tensor_mul(
                        HT[hi][p0:p0 + 64, :], os_[0:64, :], rec[:, :])

            # ================= Phase C: output projection =================
            with (
                tc.tile_pool(name="yout", bufs=2) as yout,
                tc.tile_pool(name="ps_y", bufs=2, space="PSUM") as ps_y,
            ):
                for tt in range(QT_TILES):
                    ps = ps_y.tile([128, D], FP, tag="py", name="py")
                    for c in range(2):
                        sl = slice(c * 512, (c + 1) * 512)
                        for ct in range(8):
                            nc.tensor.matmul(
                                ps[:, sl], HT[ct][:, tt * 128:(tt + 1) * 128], WO[ct][:, sl],
                                start=(ct == 0), stop=(ct == 7))
                    ytile = yout.tile([128, D], BF, tag="y", name="y")
                    nc.scalar.copy(ytile[:, :], ps[:, :])
                    nc.sync.dma_start(out=yd[tt * 128:(tt + 1) * 128, :], in_=ytile[:, :])

    nc.compile()
    canon = _canonicalize_bir(nc.to_json_bytes())
    nc.to_json_bytes = lambda: canon
    return nc


# ---------------------------------------------------------------------------
# host side: build per-core blobs, cached jit dispatch, device-resident cache
# ---------------------------------------------------------------------------

_ST = None


class _State:
    def __init__(self):
        self.nc = build_nc()
        bass2jax.install_neuronx_cc_hook()
        nc = self.nc
        partition_name = nc.partition_id_tensor.name if nc.partition_id_tensor else None
        in_names, out_names, out_avals = [], [], []
        for alloc in nc.m.functions[0].allocations:
            if not isinstance(alloc, mybir.MemoryLocationSet):
                continue
            name = alloc.memorylocations[0].name
            if alloc.kind == "ExternalInput":
                if name != partition_name:
                    in_names.append(name)
            elif alloc.kind == "ExternalOutput":
                out_names.append(name)
                out_avals.append(jax.core.ShapedArray(
                    tuple(alloc.tensor_shape), mybir.dt.np(alloc.dtype)))
        self.in_names = in_names
        bind_names = in_names + ([partition_name] if partition_name else [])

        def _body(*args):
            operands = list(args)
            if partition_name:
                operands.append(bass2jax.partition_id_tensor())
            return tuple(bass2jax._bass_exec_p.bind(
                *operands,
                out_avals=tuple(out_avals),
                in_names=tuple(bind_names),
                out_names=tuple(out_names),
                lowering_input_output_aliases=(),
                sim_require_finite=True,
                sim_require_nnan=True,
                nc=nc,
            ))

        devices = jax.devices()[:NCORES]
        self.mesh = Mesh(np.asarray(devices), ("core",))
        self.sharding = NamedSharding(self.mesh, PartitionSpec("core"))
        in_specs = (PartitionSpec("core"),) * len(in_names)
        out_specs = (PartitionSpec("core"),) * len(out_names)
        self.fn = jax.jit(jax.shard_map(
            _body, mesh=self.mesh, in_specs=in_specs, out_specs=out_specs,
            check_vma=False))
        # key -> (digest, device array)
        self.cache = {}

    def put(self, key, srcs, digest, builder):
        ent = self.cache.get(key)
        if ent is not None and ent[0] == digest:
            return ent[1]
        arr = jax.device_put(builder(), self.sharding)
        self.cache[key] = (digest, arr)
        return arr


def _digest(*arrays):
    h = hashlib.sha256()
    for a in arrays:
        h.update(np.ascontiguousarray(a).view(np.uint8).data)
    return h.digest()


def _build_xb(x_q, x_k, x_v):
    xb = np.empty((NCORES, XBN), BF16)
    for b in range(B):
        xqb = x_q[b].astype(BF16)
        xkb = x_k[b].astype(BF16).reshape(-1)
        xvb = x_v[b].astype(BF16).reshape(-1)
        for th in range(2):
            c = 2 * b + th
            xb[c, XQ_OFF:XK_OFF] = xqb[th::2].reshape(-1)
            xb[c, XK_OFF:XV_OFF] = xkb
            xb[c, XV_OFF:] = xvb
    return xb.reshape(NCORES * XBN)


def _build_wb(Wq, Wk, Wv, Wo):
    one = np.concatenate([
        Wq.T.astype(BF16).reshape(-1), Wk.T.astype(BF16).reshape(-1),
        Wv.T.astype(BF16).reshape(-1), Wo.T.astype(BF16).reshape(-1)])
    return np.tile(one, NCORES)


def _build_tb(pos_emb):
    identf = np.eye(128, dtype=np.float32)
    # SAT[p, j]: col 0 = pos_emb[0, p%64], col 1 = pos_emb[256, p%64]
    satm = np.stack([np.tile(pos_emb[0], 2), np.tile(pos_emb[2 * L], 2)], axis=1)
    tb = np.empty((NCORES, TBN), np.float32)
    for th in range(2):
        idx = np.clip(np.arange(EW) - C0 - th, -L, L) + L
        ttab = pos_emb[idx]                       # (EW, 64)
        tbl = np.concatenate([ttab.T, ttab.T], axis=0)  # (128, EW)
        blob = np.concatenate([tbl.reshape(-1), satm.reshape(-1), identf.reshape(-1)])
        for b in range(B):
            tb[2 * b + th] = blob
    return tb.reshape(NCORES * TBN)


def _build_mbk(mask):
    mb = np.where(mask[:, 0, 0, :], np.float32(NEG), np.float32(0.0)).astype(np.float32)
    return np.repeat(mb, 2, axis=0).reshape(NCORES * T)  # (B,T) -> core order


def _build_cb():
    one = np.concatenate([
        np.eye(128, dtype=BF16).reshape(-1),
        np.ones(128 * 16, BF16)])
    return np.tile(one, NCORES)


def kernel(x_q, x_k, x_v, mask, Wq, Wk, Wv, Wo, pos_emb, _trace=False, _raw=False):
    global _ST
    x_q, x_k, x_v = (np.asarray(a, np.float32) for a in (x_q, x_k, x_v))
    Wq, Wk, Wv, Wo = (np.asarray(a, np.float32) for a in (Wq, Wk, Wv, Wo))
    pos_emb = np.asarray(pos_emb, np.float32)
    mask = np.asarray(mask)

    if _ST is None:
        _ST = _State()
    st = _ST

    groups = {
        "xb": ((x_q, x_k, x_v), lambda: _build_xb(x_q, x_k, x_v)),
        "wb": ((Wq, Wk, Wv, Wo), lambda: _build_wb(Wq, Wk, Wv, Wo)),
        "tb": ((pos_emb,), lambda: _build_tb(pos_emb)),
        "mbk": ((mask,), lambda: _build_mbk(mask)),
        "cb": ((), _build_cb),
    }

    def launch():
        args = {k: st.cache[k][1] for k in groups}
        return st.fn(*[args[n] for n in st.in_names])

    def fetch_y(out):
        # Write each device shard straight into the interleaved fp32 result,
        # skipping the 8 MB bf16 global-array assembly np.asarray would do.
        y = np.empty((B, T, D), np.float32)
        for s in out[0].addressable_shards:
            c = s.index[0].start // TQ
            y[c // 2, (c % 2)::2] = np.asarray(s.data)
        return y

    if all(k in st.cache for k in groups):
        # Optimistic: launch with the cached device inputs immediately, then
        # verify content hashes while the execute + y readback is in flight
        # (hashing is fully hidden under the RPC). If anything changed,
        # rebuild the stale uploads and rerun before returning.
        out = launch()
        out[0].copy_to_host_async()  # start the y readback under the hashing
        stale = {}
        for k, (srcs, _) in groups.items():
            if srcs:
                d = _digest(*srcs)
                if d != st.cache[k][0]:
                    stale[k] = d
        y = fetch_y(out)
        if stale:
            for k, d in stale.items():
                srcs, builder = groups[k]
                st.put(k, srcs, d, builder)
            y = fetch_y(launch())
    else:
        for k, (srcs, builder) in groups.items():
            st.put(k, srcs, _digest(*srcs) if srcs else b"const", builder)
        y = fetch_y(launch())

    if _trace:
        return y, None
    return y
